# revision 37
# baseline (speedup 1.0000x reference)
"""Single-head causal attention (B=8, T=2048, C=384, H=64) on 8 NeuronCores.

Data-parallel over batch: core b computes attention for batch element b.
v3 pipeline (all matmuls bf16, fp32 PSUM):
  - host pre-transposes x -> xT chunks, packs Wqk = [Wq|Wk] per 128-chunk of C
  - QK proj: psum[0:64]=qT, psum[64:128]=kT via packed stationary (12 MMs
    N=512); vT proj via Wv stationary (12 MMs); v[s,h] blocks by PE transpose
  - qk replicated to the other partition half (SBUF->SBUF DMA) so score
    matmuls (contraction H=64) alternate PE row-groups per PSUM bank: same
    bank => same row-group (serialized -- concurrent same-bank matmuls crash
    the runtime), adjacent banks => different row-groups (run concurrently)
  - scores stream into [128,1024] PSUM windows (ring of 3); one ACTIVATE(Exp)
    per window (psum f32 -> sbuf bf16 PT); diagonals masked on GpSimd
  - output transposed: outT[h, t] += v_j[s, 0:65].T @ PT_j[s, t]; the ones
    column 64 gives the softmax denominator in row 64.  Accumulation is
    grouped per (4-strip batch, 512-col chunk) -- consecutive start..stop
    matmul groups (the only accumulation hardware honors) -- drained into an
    f32 SBUF accumulator by DVE copy/add.  Chunk groups unlock progressively
    as their own 4 strips are exp'd, so the tail after the last exp is tiny
  - normalize per quarter: PE-transpose [65,128] blocks back to [t, 65]
    (f32), 128-lane DVE reciprocal of col 64 + scale, DMA out f32
  - all work for window k is emitted at window k+1's flush so the PE queue
    never blocks on ACT; ACT table preloaded via dummy exp during input DMA
"""

import math
import os

import numpy as np
import ml_dtypes

import concourse.bass as bass
import concourse.tile as tile
from concourse import bacc, mybir
from concourse.bass import ds, ts
from concourse.bass_utils import run_bass_kernel_spmd

F32 = mybir.dt.float32
BF16 = mybir.dt.bfloat16

B, T, C, H = 8, 2048, 384, 64
P = 128
NT = T // P          # 16 key/query blocks
NCC = C // P         # 3 contraction chunks
TOTF = NT * (NT + 1) // 2 * P   # total score columns (17408)
# score windows alternate 2048/1024 columns: fewer ACTIVATE calls (each has
# a 352-cycle fixed cost) while staying double-buffered in 6 PSUM banks
WSIZES = [2048, 1024] * 5 + [2048]
WSTARTS = [sum(WSIZES[:i]) for i in range(len(WSIZES) + 1)]
assert WSTARTS[-1] == TOTF


def _win_of(fill):
    for wi in range(len(WSIZES)):
        if fill < WSTARTS[wi + 1]:
            return wi, fill - WSTARTS[wi]
    raise ValueError(fill)
SCALE = 1.0 / math.sqrt(float(C))

LAST_RESULT = None
_PROGRAM = None


def _score_chunks():
    """Yield (j, t0, w, fill) for the score chunk stream.

    Strips sequential (j = 0..15), chunks crossing neither a fill-512 (PSUM
    bank) nor a t-512 boundary.
    """
    fill = 0
    for j in range(NT):
        t = P * j
        while t < T:
            w = min(512 - fill % 512, 512 - t % 512, T - t)
            yield (j, t, w, fill)
            t += w
            fill += w


def _emit(tc: tile.TileContext, xT_d, wqk_d, wv_d, mask_d, ident_d,
          out_d, ctx, dbg_d=None):
    nc = tc.nc
    Exp = mybir.ActivationFunctionType.Exp

    sb = ctx.enter_context(tc.tile_pool(name="sb", bufs=1))
    ps = ctx.enter_context(tc.tile_pool(name="ps", bufs=1, space="PSUM"))

    # ---- sbuf tiles -------------------------------------------------------
    wqk_sb = sb.tile([P, NCC, P], BF16, tag="wqk")
    wv_sb = sb.tile([P, NCC, H], BF16, tag="wv")
    mask_sb = sb.tile([P, P], BF16, tag="mask")
    xTt = sb.tile([P, 4, NCC, 512], BF16, tag="xTt")
    qk_nat = sb.tile([P, T], BF16, tag="qk_nat")   # q in rows 0:64, k in 64:128
    qk_swp = sb.tile([P, T], BF16, tag="qk_swp")   # k in rows 0:64, q in 64:128
    vTsb = sb.tile([H, T], BF16, tag="vTsb")
    v_sb = sb.tile([P, NT, H + 1], BF16, tag="v_sb")
    ident = sb.tile([P, P], BF16, tag="ident")
    pt_all = sb.tile([P, TOTF], BF16, tag="pt_all")
    outd = sb.tile([H + 1, T], BF16, tag="outd")   # transposed out accumulator
    dum = sb.tile([1, 8], BF16, tag="dum")
    dum2 = sb.tile([1, 8], BF16, tag="dum2")
    warm = sb.tile([P, 512], BF16, tag="warm")

    def xTc(c, t4):
        return xTt[:, t4, c, :]

    # ---- ACT table preload first (1.3us DMA into ACT table RAM, runs
    # while the input DMAs stream) ------------------------------------------
    nc.vector.memset(dum[:], 0.0)
    nc.scalar.activation(dum2[:], dum[:], Exp, scale=SCALE)
    nc.vector.memset(v_sb[:, :, H], 1.0)
    nc.vector.memset(warm[:], 0.0)

    # ---- input DMAs, split across the two DGE queues (sync + scalar).
    # Each x chunk is one contiguous 3KB-per-partition run (small-descriptor
    # destination patterns run the DMA at ~60 GB/s instead of ~350)
    nc.sync.dma_start(wqk_sb[:], wqk_d[:])
    nc.scalar.dma_start(wv_sb[:], wv_d[:])
    nc.sync.dma_start(xTt[:, 0], xT_d[0])
    nc.scalar.dma_start(xTt[:, 1], xT_d[1])
    nc.sync.dma_start(xTt[:, 2], xT_d[2])
    nc.scalar.dma_start(xTt[:, 3], xT_d[3])
    nc.scalar.dma_start(mask_sb[:], mask_d[:])
    nc.scalar.dma_start(ident[:], ident_d[:])

    # PE warm-up while the input DMAs stream: HAM starts throttled at
    # 1.2 GHz and needs ~3.4us of sustained array activity to unthrottle
    wp = ps.tile([P, 512], F32, tag="acc", bufs=2, name="warm_ps")
    for _ in range(11):
        nc.tensor.matmul(wp[:], warm[:, 0:P], warm[:], start=True, stop=True)

    def emit_vtr(j):
        # v block j via PE transpose (XBAR DMA transposes cost ~1.2us each
        # on a DGE queue -- way too slow)
        tr = ps.tile([P, H], BF16, tag="acc", bufs=2, name=f"vtr{j}")
        nc.tensor.transpose(tr[:], vTsb[:, ds(P * j, P)], ident[0:H, 0:H])
        nc.vector.tensor_copy(v_sb[:, j, 0:H], tr[:])

    # ---- projections for one 512-col t-chunk ------------------------------
    proj_tiles = {}

    def emit_proj(t4):
        if t4 % 2 == 0:
            proj_tiles[t4 // 2] = ps.tile(
                [P, 2048], F32, tag="winA", bufs=1, name=f"proj{t4}")
        w0 = proj_tiles[t4 // 2]
        w = w0[:, ds(1024 * (t4 % 2), 1024)]
        for c in range(NCC):
            nc.tensor.matmul(
                w[:, 0:512], wqk_sb[:, c, :], xTc(c, t4),
                start=(c == 0), stop=(c == NCC - 1),
            )
        for c in range(NCC):
            nc.tensor.matmul(
                w[0:H, 512:1024], wv_sb[:, c, :], xTc(c, t4),
                start=(c == 0), stop=(c == NCC - 1),
            )
        nc.vector.tensor_copy(qk_nat[:, ts(t4, 512)], w[:, 0:512])
        nc.vector.tensor_copy(vTsb[:, ts(t4, 512)], w[0:H, 512:1024])
        # replicate to the other partition half (k -> low, q -> high)
        nc.sync.dma_start(qk_swp[0:H, ts(t4, 512)], qk_nat[H:P, ts(t4, 512)])
        nc.sync.dma_start(qk_swp[H:P, ts(t4, 512)], qk_nat[0:H, ts(t4, 512)])

    # ---- main loop --------------------------------------------------------
    # score operands by row-group: rows 0:64 = (k from swp, q from nat),
    # rows 64:128 = (k from nat, q from swp)
    qA, kA = qk_nat[0:H, :], qk_swp[0:H, :]
    qB, kB = qk_swp[H:P, :], qk_nat[H:P, :]

    out_v = out_d.rearrange("(g i p) h -> g p i h", p=P, i=4)

    all_chunks = list(_score_chunks())
    # pt layout: strip j occupies pt_all[:, strip_base[j] : +T-128j] contiguous
    strip_base = {}
    for (j, t0, w, fill) in all_chunks:
        if j not in strip_base:
            strip_base[j] = fill

    # outT work units: (batch b of strips 4b..4b+3, 512-col chunk q >= b).
    # Unlock window = when all 4 strips' pt covers t < 512(q+1).
    units = []
    for b in range(4):
        for q in range(b, 4):
            need = max(
                strip_base[j] + 512 * (q + 1) - P * j
                for j in range(4 * b, 4 * b + 4)
            )
            units.append((_win_of(need - 1)[0], b, q))
    units.sort()
    q_parts_done = [0] * 4

    win_tiles = {}
    pending = []              # chunks of the newest un-exped window

    def emit_unit(b, q):
        # one consecutive accumulation group: strips 4b..4b+3 into out cols
        # [512q, 512q+512); strips entering mid-chunk join at partial width
        oa = ps.tile([P, 512], F32, tag="acc", bufs=2, name=f"u{b}_{q}")
        js = list(range(4 * b, 4 * b + 4))
        for n, j in enumerate(js):
            lo = max(512 * q, P * j)
            nc.tensor.matmul(
                oa[0:H + 1, ds(lo - 512 * q, 512 * (q + 1) - lo)],
                v_sb[:, j, 0:H + 1],
                pt_all[:, ds(strip_base[j] + lo - P * j, 512 * (q + 1) - lo)],
                start=(n == 0), stop=(n == len(js) - 1),
                skip_group_check=True,
            )
        if b == 0:
            nc.vector.tensor_copy(outd[0:H + 1, ts(q, 512)], oa[0:H + 1, :])
        else:
            nc.vector.tensor_add(
                outd[0:H + 1, ts(q, 512)], outd[0:H + 1, ts(q, 512)],
                oa[0:H + 1, :],
            )
        q_parts_done[q] += 1
        if q_parts_done[q] == q + 1:
            emit_qnorm(q)

    def emit_qnorm(q):
        # normalize quarter q: PE-transpose each 128-block back to [t, 65]
        # (f32), then 128-lane reciprocal + scale on DVE (a single-partition
        # reciprocal on the denominator row costs 3.3us -- never do that)
        outf = sb.tile([P, 4, H], F32, tag="outf", bufs=2, name=f"outf{q}")
        r = sb.tile([P, 4], F32, tag="recip", bufs=2, name=f"recip{q}")
        for bb in range(4):
            tr = ps.tile([P, H + 1], BF16, tag="acc", bufs=2,
                         name=f"otr{q}_{bb}")
            nc.tensor.transpose(
                tr[:], outd[:, ds(512 * q + P * bb, P)],
                ident[0:H + 1, 0:H + 1]
            )
            nc.vector.reciprocal(r[:, ds(bb, 1)], tr[:, H:H + 1])
            nc.vector.tensor_scalar_mul(outf[:, bb, :], tr[:, 0:H],
                                        r[:, ds(bb, 1)])
        nc.sync.dma_start(out_v[q], outf[:])

    def flush(wid):
        # exp the filled window; then (while ACT runs) masks, v transposes,
        # and any outT unit groups whose strips are now all exp'd
        nonlocal pending
        if not pending:
            return
        wt, fill = win_tiles.pop(wid)
        pt0 = WSTARTS[wid]
        nc.scalar.activation(pt_all[:, ds(pt0, fill)], wt[:, 0:fill], Exp,
                             scale=SCALE)
        for (j, t0, w, fpos) in pending:
            pt_off = pt0 + fpos
            # mask any part of this chunk inside the strip's diagonal block
            dlo, dhi = P * j, P * j + P
            mlo, mhi = max(t0, dlo), min(t0 + w, dhi)
            if mlo < mhi:
                nc.gpsimd.tensor_mul(
                    pt_all[:, ds(pt_off + (mlo - t0), mhi - mlo)],
                    pt_all[:, ds(pt_off + (mlo - t0), mhi - mlo)],
                    mask_sb[:, ds(mlo - dlo, mhi - mlo)],
                )
        pending = []

    emit_proj(0)
    emit_proj(1)
    emit_proj(2)
    emit_proj(3)
    cur_wid = 0
    for (j, t0, w, fill) in all_chunks:
        wid, fpos = _win_of(fill)
        if wid != cur_wid:
            flush(cur_wid)
            cur_wid = wid
        if fpos == 0:
            wsz = WSIZES[wid]
            wt = ps.tile([P, wsz], F32,
                         tag="winA" if wsz == 2048 else "winB",
                         bufs=1, name=f"win{wid}")
            win_tiles[wid] = (wt, 0)
        wt, wfill = win_tiles[wid]
        assert wfill == fpos, (wfill, fpos)
        rg = (fill // 512) % 2
        stat = kA if rg == 0 else kB
        mov = qA if rg == 0 else qB
        nc.tensor.matmul(
            wt[:, ds(fpos, w)],
            stat[:, ds(P * j, P)],
            mov[:, ds(t0, w)],
            start=True, stop=True,
        )
        win_tiles[wid] = (wt, wfill + w)
        pending.append((j, t0, w, fpos))
    flush(cur_wid)
    # lower-priority filler work: the Tile scheduler slots these into PE
    # gaps as their dependencies (projections / exps / masks) resolve
    for j in range(NT):
        emit_vtr(j)
    for (_w, b, q) in units:
        emit_unit(b, q)
    if dbg_d is not None:
        nc.sync.dma_start(dbg_d[:, 0:NT * (H + 1)],
                          v_sb.rearrange("p j h -> p (j h)"))
        nc.sync.dma_start(dbg_d[:, 2048:2048 + 4096],
                          pt_all[:, 0:4096])


def _build_program(num_devices=B, debug_out=False):
    nc = bacc.Bacc("TRN2", target_bir_lowering=False, debug=False,
                   num_devices=num_devices)
    xT_d = nc.dram_tensor("xT", [4, P, NCC, 512], BF16,
                          kind="ExternalInput").ap()
    wqk_d = nc.dram_tensor("wqk", [P, NCC, P], BF16, kind="ExternalInput").ap()
    wv_d = nc.dram_tensor("wv", [P, NCC, H], BF16, kind="ExternalInput").ap()
    mask_d = nc.dram_tensor("mask", [P, P], BF16, kind="ExternalInput").ap()
    ident_d = nc.dram_tensor("ident", [P, P], BF16, kind="ExternalInput").ap()
    out_d = nc.dram_tensor("out", [T, H], F32, kind="ExternalOutput").ap()
    dbg_d = None
    if debug_out:
        dbg_d = nc.dram_tensor("dbg", [P, 8192], BF16,
                               kind="ExternalOutput").ap()
    from contextlib import ExitStack

    with tile.TileContext(nc) as tc:
        with ExitStack() as ctx:
            _emit(tc, xT_d, wqk_d, wv_d, mask_d, ident_d,
                  out_d, ctx, dbg_d=dbg_d)
    nc.compile()
    return nc


def _host_inputs(x, Wq, Wk, Wv):
    bf = ml_dtypes.bfloat16
    xT = np.ascontiguousarray(np.transpose(x, (0, 2, 1))).astype(bf)
    Bn = x.shape[0]
    # xT: [t4, 128, c, 512] -- one contiguous run per (partition, t4)
    xTr = xT.reshape(Bn, NCC, P, 4, 512)
    xTn = np.ascontiguousarray(xTr.transpose(0, 3, 2, 1, 4))
    wqk = np.concatenate([Wq, Wk], axis=1).reshape(NCC, P, 2 * H)
    wqk = np.ascontiguousarray(np.transpose(wqk, (1, 0, 2))).astype(bf)
    wv = np.ascontiguousarray(
        np.transpose(Wv.reshape(NCC, P, H), (1, 0, 2))
    ).astype(bf)
    # mask[s, t] = 1 where s <= t (transposed-causal diagonal block)
    mask = np.triu(np.ones((P, P), dtype=np.float32)).astype(bf)
    identity = np.eye(P, dtype=np.float32).astype(bf)
    return xTn, wqk, wv, mask, identity


def kernel(x, Wq, Wk, Wv):
    global LAST_RESULT, _PROGRAM
    assert x.shape == (B, T, C), x.shape
    if _PROGRAM is None:
        _PROGRAM = _build_program()
    nc = _PROGRAM

    xTn, wqk, wv, mask, identity = _host_inputs(x, Wq, Wk, Wv)
    in_maps = [
        {"xT": xTn[b], "wqk": wqk, "wv": wv, "mask": mask, "ident": identity}
        for b in range(B)
    ]
    trace = bool(int(os.environ.get("KERNEL_TRACE", "0")))
    kw = {}
    td = os.environ.get("KERNEL_TRACE_DIR")
    if td:
        kw["tmpdir"] = td
    LAST_RESULT = run_bass_kernel_spmd(
        nc, in_maps, list(range(B)), trace=trace, **kw
    )
    out = np.stack([LAST_RESULT.results[b]["out"] for b in range(B)], axis=0)
    return out.astype(np.float32)


# revision 38
# speedup vs baseline: 1.2027x; 1.2027x over previous
"""Single-head causal attention (B=8, T=2048, C=384, H=64) on 8 NeuronCores.

Data-parallel over batch: core b computes attention for batch element b.
v3 pipeline (all matmuls bf16, fp32 PSUM):
  - host pre-transposes x -> xT chunks, packs Wqk = [Wq|Wk] per 128-chunk of C
  - QK proj: psum[0:64]=qT, psum[64:128]=kT via packed stationary (12 MMs
    N=512); vT proj via Wv stationary (12 MMs); v[s,h] blocks by PE transpose
  - qk replicated to the other partition half (SBUF->SBUF DMA) so score
    matmuls (contraction H=64) alternate PE row-groups per PSUM bank: same
    bank => same row-group (serialized -- concurrent same-bank matmuls crash
    the runtime), adjacent banks => different row-groups (run concurrently)
  - scores stream into [128,1024] PSUM windows (ring of 3); one ACTIVATE(Exp)
    per window (psum f32 -> sbuf bf16 PT); diagonals masked on GpSimd
  - output transposed: outT[h, t] += v_j[s, 0:65].T @ PT_j[s, t]; the ones
    column 64 gives the softmax denominator in row 64.  Accumulation is
    grouped per (4-strip batch, 512-col chunk) -- consecutive start..stop
    matmul groups (the only accumulation hardware honors) -- drained into an
    f32 SBUF accumulator by DVE copy/add.  Chunk groups unlock progressively
    as their own 4 strips are exp'd, so the tail after the last exp is tiny
  - normalize per quarter: PE-transpose [65,128] blocks back to [t, 65]
    (f32), 128-lane DVE reciprocal of col 64 + scale, DMA out f32
  - all work for window k is emitted at window k+1's flush so the PE queue
    never blocks on ACT; ACT table preloaded via dummy exp during input DMA
"""

import math
import os

import numpy as np
import ml_dtypes

import concourse.bass as bass
import concourse.tile as tile
from concourse import bacc, mybir
from concourse.bass import ds, ts
from concourse.bass_utils import run_bass_kernel_spmd

F32 = mybir.dt.float32
BF16 = mybir.dt.bfloat16

B, T, C, H = 8, 2048, 384, 64
P = 128
NT = T // P          # 16 key/query blocks
NCC = C // P         # 3 contraction chunks
WIN = 1024           # score window columns (2 PSUM banks)
TOTF = NT * (NT + 1) // 2 * P   # total score columns (17408)
SCALE = 1.0 / math.sqrt(float(C))

LAST_RESULT = None
_PROGRAM = None


def _score_chunks():
    """Yield (j, t0, w, fill) for the score chunk stream.

    Strips sequential (j = 0..15), chunks crossing neither a fill-512 (PSUM
    bank) nor a t-512 boundary.
    """
    fill = 0
    for j in range(NT):
        t = P * j
        while t < T:
            w = min(512 - fill % 512, 512 - t % 512, T - t)
            yield (j, t, w, fill)
            t += w
            fill += w


def _emit(tc: tile.TileContext, xT_d, wqk_d, wv_d, mask_d, ident_d,
          out_d, ctx, dbg_d=None):
    nc = tc.nc
    Exp = mybir.ActivationFunctionType.Exp

    sb = ctx.enter_context(tc.tile_pool(name="sb", bufs=1))
    ps = ctx.enter_context(tc.tile_pool(name="ps", bufs=1, space="PSUM"))

    # ---- sbuf tiles -------------------------------------------------------
    wqk_sb = sb.tile([P, NCC, P], BF16, tag="wqk")
    wv_sb = sb.tile([P, NCC, H], BF16, tag="wv")
    mask_sb = sb.tile([P, P], BF16, tag="mask")
    xTt = sb.tile([P, 4, NCC, 512], BF16, tag="xTt")
    qk_nat = sb.tile([P, T], BF16, tag="qk_nat")   # q in rows 0:64, k in 64:128
    qk_swp = sb.tile([P, T], BF16, tag="qk_swp")   # k in rows 0:64, q in 64:128
    vTsb = sb.tile([H, T], BF16, tag="vTsb")
    v_sb = sb.tile([P, NT, H + 1], BF16, tag="v_sb")
    ident = sb.tile([P, P], BF16, tag="ident")
    n_win = (TOTF + WIN - 1) // WIN
    pt_all = sb.tile([P, n_win * WIN], BF16, tag="pt_all")
    outd = sb.tile([H + 1, T], BF16, tag="outd")   # transposed out accumulator
    dum = sb.tile([1, 8], BF16, tag="dum")
    dum2 = sb.tile([1, 8], BF16, tag="dum2")
    warm = sb.tile([P, 512], BF16, tag="warm")

    def xTc(c, t4):
        return xTt[:, t4, c, :]

    # ---- ACT table preload first (1.3us DMA into ACT table RAM, runs
    # while the input DMAs stream) ------------------------------------------
    nc.vector.memset(dum[:], 0.0)
    nc.scalar.activation(dum2[:], dum[:], Exp, scale=SCALE)
    nc.vector.memset(v_sb[:, :, H], 1.0)
    nc.vector.memset(warm[:], 0.0)

    # ---- input DMAs, split across the two DGE queues (sync + scalar).
    # Each x chunk is one contiguous 3KB-per-partition run (small-descriptor
    # destination patterns run the DMA at ~60 GB/s instead of ~350)
    nc.sync.dma_start(wqk_sb[:], wqk_d[:])
    nc.scalar.dma_start(wv_sb[:], wv_d[:])
    nc.sync.dma_start(xTt[:, 0], xT_d[0])
    nc.scalar.dma_start(xTt[:, 1], xT_d[1])
    nc.sync.dma_start(xTt[:, 2], xT_d[2])
    nc.scalar.dma_start(xTt[:, 3], xT_d[3])
    nc.scalar.dma_start(mask_sb[:], mask_d[:])
    nc.scalar.dma_start(ident[:], ident_d[:])

    # PE warm-up while the input DMAs stream: HAM starts throttled at
    # 1.2 GHz and needs ~3.4us of sustained array activity to unthrottle
    wp = ps.tile([P, 512], F32, tag="acc", bufs=4, name="warm_ps")
    for _ in range(11):
        nc.tensor.matmul(wp[:], warm[:, 0:P], warm[:], start=True, stop=True)

    def emit_vtr(j):
        # v block j via PE transpose (XBAR DMA transposes cost ~1.2us each
        # on a DGE queue -- way too slow)
        tr = ps.tile([P, H], BF16, tag="acc", bufs=4, name=f"vtr{j}")
        nc.tensor.transpose(tr[:], vTsb[:, ds(P * j, P)], ident[0:H, 0:H])
        nc.vector.tensor_copy(v_sb[:, j, 0:H], tr[:])

    # ---- projections for one 512-col t-chunk ------------------------------
    def emit_proj(t4):
        w = ps.tile([P, WIN], F32, tag="win", bufs=2, name=f"proj{t4}")
        for c in range(NCC):
            nc.tensor.matmul(
                w[:, 0:512], wqk_sb[:, c, :], xTc(c, t4),
                start=(c == 0), stop=(c == NCC - 1),
            )
        for c in range(NCC):
            nc.tensor.matmul(
                w[0:H, 512:1024], wv_sb[:, c, :], xTc(c, t4),
                start=(c == 0), stop=(c == NCC - 1),
            )
        nc.vector.tensor_copy(qk_nat[:, ts(t4, 512)], w[:, 0:512])
        nc.vector.tensor_copy(vTsb[:, ts(t4, 512)], w[0:H, 512:1024])
        # replicate to the other partition half (k -> low, q -> high)
        nc.sync.dma_start(qk_swp[0:H, ts(t4, 512)], qk_nat[H:P, ts(t4, 512)])
        nc.sync.dma_start(qk_swp[H:P, ts(t4, 512)], qk_nat[0:H, ts(t4, 512)])

    # ---- main loop --------------------------------------------------------
    # score operands by row-group: rows 0:64 = (k from swp, q from nat),
    # rows 64:128 = (k from nat, q from swp)
    qA, kA = qk_nat[0:H, :], qk_swp[0:H, :]
    qB, kB = qk_swp[H:P, :], qk_nat[H:P, :]

    out_v = out_d.rearrange("(g i p) h -> g p i h", p=P, i=4)

    all_chunks = list(_score_chunks())
    # pt layout: strip j occupies pt_all[:, strip_base[j] : +T-128j] contiguous
    strip_base = {}
    for (j, t0, w, fill) in all_chunks:
        if j not in strip_base:
            strip_base[j] = fill

    # outT work units: (batch b of strips 4b..4b+3, 512-col chunk q >= b).
    # Unlock window = when all 4 strips' pt covers t < 512(q+1).
    units = []
    for b in range(4):
        for q in range(b, 4):
            need = max(
                strip_base[j] + 512 * (q + 1) - P * j
                for j in range(4 * b, 4 * b + 4)
            )
            units.append((min((need - 1) // WIN, n_win - 1), b, q))
    units.sort()
    q_parts_done = [0] * 4

    win_tiles = {}
    pending = []              # chunks of the newest un-exped window

    def emit_unit(b, q):
        # one consecutive accumulation group: strips 4b..4b+3 into out cols
        # [512q, 512q+512); strips entering mid-chunk join at partial width
        oa = ps.tile([P, 512], F32, tag="acc", bufs=4, name=f"u{b}_{q}")
        js = list(range(4 * b, 4 * b + 4))
        for n, j in enumerate(js):
            lo = max(512 * q, P * j)
            nc.tensor.matmul(
                oa[0:H + 1, ds(lo - 512 * q, 512 * (q + 1) - lo)],
                v_sb[:, j, 0:H + 1],
                pt_all[:, ds(strip_base[j] + lo - P * j, 512 * (q + 1) - lo)],
                start=(n == 0), stop=(n == len(js) - 1),
                skip_group_check=True,
            )
        if b == 0:
            nc.vector.tensor_copy(outd[0:H + 1, ts(q, 512)], oa[0:H + 1, :])
        else:
            nc.vector.tensor_add(
                outd[0:H + 1, ts(q, 512)], outd[0:H + 1, ts(q, 512)],
                oa[0:H + 1, :],
            )
        q_parts_done[q] += 1
        if q_parts_done[q] == q + 1:
            emit_qnorm(q)

    def emit_qnorm(q):
        # normalize quarter q: PE-transpose each 128-block back to [t, 65]
        # (f32), then 128-lane reciprocal + scale on DVE (a single-partition
        # reciprocal on the denominator row costs 3.3us -- never do that)
        outf = sb.tile([P, 4, H], F32, tag="outf", bufs=2, name=f"outf{q}")
        r = sb.tile([P, 4], F32, tag="recip", bufs=2, name=f"recip{q}")
        for bb in range(4):
            tr = ps.tile([P, H + 1], BF16, tag="acc", bufs=4,
                         name=f"otr{q}_{bb}")
            nc.tensor.transpose(
                tr[:], outd[:, ds(512 * q + P * bb, P)],
                ident[0:H + 1, 0:H + 1]
            )
            nc.vector.reciprocal(r[:, ds(bb, 1)], tr[:, H:H + 1])
            nc.vector.tensor_scalar_mul(outf[:, bb, :], tr[:, 0:H],
                                        r[:, ds(bb, 1)])
        nc.sync.dma_start(out_v[q], outf[:])

    def flush(wid):
        # exp the filled window; then (while ACT runs) masks, v transposes,
        # and any outT unit groups whose strips are now all exp'd
        nonlocal pending
        if not pending:
            return
        wt, fill = win_tiles.pop(wid)
        pt0 = wid * WIN
        nc.scalar.activation(pt_all[:, ds(pt0, fill)], wt[:, 0:fill], Exp,
                             scale=SCALE)
        for (j, t0, w, fpos) in pending:
            pt_off = pt0 + fpos
            # mask any part of this chunk inside the strip's diagonal block
            dlo, dhi = P * j, P * j + P
            mlo, mhi = max(t0, dlo), min(t0 + w, dhi)
            if mlo < mhi:
                nc.gpsimd.tensor_mul(
                    pt_all[:, ds(pt_off + (mlo - t0), mhi - mlo)],
                    pt_all[:, ds(pt_off + (mlo - t0), mhi - mlo)],
                    mask_sb[:, ds(mlo - dlo, mhi - mlo)],
                )
        pending = []

    emit_proj(0)
    emit_proj(1)
    emit_proj(2)
    emit_proj(3)
    cur_wid = 0
    for (j, t0, w, fill) in all_chunks:
        wid, fpos = fill // WIN, fill % WIN
        if wid != cur_wid:
            flush(cur_wid)
            cur_wid = wid
        if fpos == 0:
            wt = ps.tile([P, WIN], F32, tag="win", bufs=2, name=f"win{wid}")
            win_tiles[wid] = (wt, 0)
        wt, wfill = win_tiles[wid]
        assert wfill == fpos, (wfill, fpos)
        rg = (fill // 512) % 2
        stat = kA if rg == 0 else kB
        mov = qA if rg == 0 else qB
        nc.tensor.matmul(
            wt[:, ds(fpos, w)],
            stat[:, ds(P * j, P)],
            mov[:, ds(t0, w)],
            start=True, stop=True,
        )
        win_tiles[wid] = (wt, wfill + w)
        pending.append((j, t0, w, fpos))
    flush(cur_wid)
    # lower-priority filler work: the Tile scheduler slots these into PE
    # gaps as their dependencies (projections / exps / masks) resolve
    for j in range(NT):
        emit_vtr(j)
    for (_w, b, q) in units:
        emit_unit(b, q)
    if dbg_d is not None:
        nc.sync.dma_start(dbg_d[:, 0:NT * (H + 1)],
                          v_sb.rearrange("p j h -> p (j h)"))
        nc.sync.dma_start(dbg_d[:, 2048:2048 + 4096],
                          pt_all[:, 0:4096])


def _build_program(num_devices=B, debug_out=False):
    nc = bacc.Bacc("TRN2", target_bir_lowering=False, debug=False,
                   num_devices=num_devices)
    xT_d = nc.dram_tensor("xT", [4, P, NCC, 512], BF16,
                          kind="ExternalInput").ap()
    wqk_d = nc.dram_tensor("wqk", [P, NCC, P], BF16, kind="ExternalInput").ap()
    wv_d = nc.dram_tensor("wv", [P, NCC, H], BF16, kind="ExternalInput").ap()
    mask_d = nc.dram_tensor("mask", [P, P], BF16, kind="ExternalInput").ap()
    ident_d = nc.dram_tensor("ident", [P, P], BF16, kind="ExternalInput").ap()
    out_d = nc.dram_tensor("out", [T, H], F32, kind="ExternalOutput").ap()
    dbg_d = None
    if debug_out:
        dbg_d = nc.dram_tensor("dbg", [P, 8192], BF16,
                               kind="ExternalOutput").ap()
    from contextlib import ExitStack

    with tile.TileContext(nc) as tc:
        with ExitStack() as ctx:
            _emit(tc, xT_d, wqk_d, wv_d, mask_d, ident_d,
                  out_d, ctx, dbg_d=dbg_d)
    nc.compile()
    return nc


def _host_inputs(x, Wq, Wk, Wv):
    bf = ml_dtypes.bfloat16
    xT = np.ascontiguousarray(np.transpose(x, (0, 2, 1))).astype(bf)
    Bn = x.shape[0]
    # xT: [t4, 128, c, 512] -- one contiguous run per (partition, t4)
    xTr = xT.reshape(Bn, NCC, P, 4, 512)
    xTn = np.ascontiguousarray(xTr.transpose(0, 3, 2, 1, 4))
    wqk = np.concatenate([Wq, Wk], axis=1).reshape(NCC, P, 2 * H)
    wqk = np.ascontiguousarray(np.transpose(wqk, (1, 0, 2))).astype(bf)
    wv = np.ascontiguousarray(
        np.transpose(Wv.reshape(NCC, P, H), (1, 0, 2))
    ).astype(bf)
    # mask[s, t] = 1 where s <= t (transposed-causal diagonal block)
    mask = np.triu(np.ones((P, P), dtype=np.float32)).astype(bf)
    identity = np.eye(P, dtype=np.float32).astype(bf)
    return xTn, wqk, wv, mask, identity


def kernel(x, Wq, Wk, Wv):
    global LAST_RESULT, _PROGRAM
    assert x.shape == (B, T, C), x.shape
    if _PROGRAM is None:
        _PROGRAM = _build_program()
    nc = _PROGRAM

    xTn, wqk, wv, mask, identity = _host_inputs(x, Wq, Wk, Wv)
    in_maps = [
        {"xT": xTn[b], "wqk": wqk, "wv": wv, "mask": mask, "ident": identity}
        for b in range(B)
    ]
    trace = bool(int(os.environ.get("KERNEL_TRACE", "0")))
    kw = {}
    td = os.environ.get("KERNEL_TRACE_DIR")
    if td:
        kw["tmpdir"] = td
    LAST_RESULT = run_bass_kernel_spmd(
        nc, in_maps, list(range(B)), trace=trace, **kw
    )
    out = np.stack([LAST_RESULT.results[b]["out"] for b in range(B)], axis=0)
    return out.astype(np.float32)
